# revision 1
# baseline (speedup 1.0000x reference)
"""Trainium2 Bass kernel for nn_CHESHIRE (hypergraph GNN message passing).

Strategy (hyperedge-parallel across the 8 cores):
  * The clique-edge structure is a disjoint union of 8-node cliques (one per
    hyperedge), so the normalized Laplacian has the closed form
    lap(v) = (v - group_sum(v)) / 7 and the K=3 Chebyshev conv collapses to
    out = x_gn @ Wx + gsum(x_gn) @ Wg with host-folded weight combos.
  * GraphNorm is a per-hyperedge affine x_gn = x*A_e + B_e folded into the
    same matmuls; only per-hyperedge [EMB] stats are computed on device.
  * Node encodings (and their squares) are computed once per core and stored
    to DRAM as an fp16 [node, x||x^2] table; incidence rows are fetched with
    per-partition indirect DMAs (128 rows each), member-plane-major so all
    per-hyperedge reductions become plane-wise ops: PE identity-matmul
    accumulation for sums, pairwise-max/min trees for the poolings.
"""

import sys

sys.path.insert(0, "/opt/trn_rl_repo")

import numpy as np

import concourse.bacc as bacc
import concourse.bass as bass
import concourse.mybir as mybir
from concourse import tile
from concourse.bass_utils import run_bass_kernel_spmd

F16 = mybir.dt.float16
F32 = mybir.dt.float32
I32 = mybir.dt.int32
AF = mybir.ActivationFunctionType
OP = mybir.AluOpType

# Problem constants (hardcoded per contract).
N, F, EMB, CONV = 2000, 256, 128, 128
E, S = 20000, 8
NCORES = 8
ECORE = E // NCORES          # 2500
EPAD = 2560                  # padded per-core edge count
NBLK = 5
L = EPAD // NBLK             # 512 edges per block
COLS = S * L                 # 4096 gathered columns per block
NG = NBLK * S * (L // 128)   # 160 gather instructions per core
# tapered blocks: long chains amortize early, short chain at the tail
_SIZES = [512, 512, 512, 512, 256, 128, 128]
BLOCKS = []
_o = 0
for _l in _SIZES:
    BLOCKS.append((_o, _l))
    _o += _l
assert _o == EPAD
NPAD = 2048                  # padded node count
EPS = 1e-5

_CACHE = {}


def _build_program():
    nc = bacc.Bacc(None, target_bir_lowering=False, debug=False)

    featT_d = nc.dram_tensor("featT", [F, NPAD], F16, kind="ExternalInput")
    wenc_d = nc.dram_tensor("wenc", [F, EMB], F16, kind="ExternalInput")
    benc_d = nc.dram_tensor("benc", [1, EMB], F16, kind="ExternalInput")
    wx_d = nc.dram_tensor("wx", [EMB, CONV], F16, kind="ExternalInput")
    wu_d = nc.dram_tensor("wu", [EMB, CONV], F16, kind="ExternalInput")
    ww_d = nc.dram_tensor("ww", [EMB, CONV], F16, kind="ExternalInput")
    wo_d = nc.dram_tensor("wo", [CONV, 2], F16, kind="ExternalInput")
    eyef_d = nc.dram_tensor("eyef", [128, 128], F16, kind="ExternalInput")
    eye32_d = nc.dram_tensor("eye32", [128, 128], F32, kind="ExternalInput")
    vecs_d = nc.dram_tensor("vecs", [128, 8], F32, kind="ExternalInput")
    idx_d = nc.dram_tensor("idx32", [128, NG], I32, kind="ExternalInput")
    yout_d = nc.dram_tensor("yout", [EPAD], F32, kind="ExternalOutput")

    xcat_d = nc.dram_tensor("xcat_scratch", [NPAD, 2 * EMB], F16)

    with tile.TileContext(nc) as tc:
        with (
            tc.tile_pool(name="weights", bufs=1) as wpool,
            tc.tile_pool(name="smalls", bufs=1) as spool,
            tc.tile_pool(name="gath", bufs=1) as gpool,
            tc.tile_pool(name="big", bufs=2) as bigp,
            tc.tile_pool(name="psA", bufs=1, space="PSUM") as psA,
            tc.tile_pool(name="psB", bufs=1, space="PSUM") as psB,
        ):
            # ---- load weights / tables ----
            featT0 = wpool.tile([128, NPAD], F16, tag="featT0")
            featT1 = wpool.tile([128, NPAD], F16, tag="featT1")
            nc.sync.dma_start(featT0[:], featT_d[0:128, :])
            nc.sync.dma_start(featT1[:], featT_d[128:256, :])
            wenc0 = wpool.tile([128, EMB], F16, tag="wenc0")
            wenc1 = wpool.tile([128, EMB], F16, tag="wenc1")
            nc.sync.dma_start(wenc0[:], wenc_d[0:128, :])
            nc.sync.dma_start(wenc1[:], wenc_d[128:256, :])
            benc = wpool.tile([1, EMB], F16, tag="benc")
            nc.sync.dma_start(benc[:], benc_d[:])
            wx = wpool.tile([EMB, CONV], F16, tag="wx")
            nc.sync.dma_start(wx[:], wx_d[:])
            wu = wpool.tile([EMB, CONV], F16, tag="wu")
            nc.sync.dma_start(wu[:], wu_d[:])
            ww = wpool.tile([EMB, CONV], F16, tag="ww")
            nc.sync.dma_start(ww[:], ww_d[:])
            wo = wpool.tile([CONV, 2], F16, tag="wo")
            nc.sync.dma_start(wo[:], wo_d[:])
            eyef = wpool.tile([128, 128], F16, tag="eyef")
            nc.sync.dma_start(eyef[:], eyef_d[:])
            eye32 = wpool.tile([128, 128], F32, tag="eye32")
            nc.sync.dma_start(eye32[:], eye32_d[:])
            vecs = wpool.tile([128, 8], F32, tag="vecs")
            nc.sync.dma_start(vecs[:], vecs_d[:])
            idx = wpool.tile([128, NG], I32, tag="idx")
            nc.sync.dma_start(idx[:], idx_d[:])
            ones = wpool.tile([1, 128], F16, tag="ones")
            nc.vector.memset(ones[:], 1.0)

            c2v = vecs[:, 0:1]     # (2s - s^2)/8
            wgv = vecs[:, 1:2]     # gn_weight
            s8v = vecs[:, 2:3]     # gn_mean_scale/8
            cconv = vecs[:, 3:4]   # c_const (+cheb_b) per CONV feature
            boutv = vecs[0:1, 4:5]  # b_out scalar

            # ---- encoder: x_enc = clip(feat @ W_enc + b_enc) -> fp16 tables
            xenc = wpool.tile([128, NPAD], F16, tag="xenc")
            xsq = wpool.tile([128, NPAD], F16, tag="xsq")
            for g in range(4):
                ep = psB.tile([128, 512], F32, tag="vp", name="ep", bufs=2)
                for t4 in range(4):
                    t = 4 * g + t4
                    sl = bass.ts(t, 128)
                    out = ep[:, bass.ts(t4, 128)]
                    nc.tensor.matmul(out, featT0[:, sl], wenc0[:],
                                     start=True, stop=False)
                    nc.tensor.matmul(out, featT1[:, sl], wenc1[:],
                                     start=False, stop=False)
                    nc.tensor.matmul(out, ones[:], benc[:],
                                     start=False, stop=True)
                nc.vector.tensor_scalar(xenc[:, bass.ts(g, 512)], ep[:],
                                        1.0, -1.0, op0=OP.min, op1=OP.max)
                nc.scalar.activation(xsq[:, bass.ts(g, 512)],
                                     xenc[:, bass.ts(g, 512)], AF.Square)
                # store this 512-node slice of the [x || x^2] table
                r0 = g * 512
                nc.sync.dma_start(
                    xcat_d[r0:r0 + 512, 0:EMB].rearrange(
                        "(t p) e -> p t e", p=128),
                    xenc[:, bass.ts(g, 512)].rearrange(
                        "p (t e) -> p t e", e=128),
                )
                nc.sync.dma_start(
                    xcat_d[r0:r0 + 512, EMB:2 * EMB].rearrange(
                        "(t p) e -> p t e", p=128),
                    xsq[:, bass.ts(g, 512)].rearrange(
                        "p (t e) -> p t e", e=128),
                )

            logit = wpool.tile([1, EPAD], F32, tag="logit")

            tcol = 0
            for b, (e0, Lb) in enumerate(BLOCKS):
                # ---- gather 4096 incidence rows (row-major, [x || x^2]) ----
                xg = []  # xg[j]: [128 edges, 4 quarters, 256] fp16
                for j in range(S):
                    g_j = gpool.tile([128, Lb // 128, 2 * EMB], F16, tag=f"xg{b}_{j}",
                                     name=f"xg{b}_{j}")
                    for q in range(Lb // 128):
                        t = tcol + j * (Lb // 128) + q
                        nc.gpsimd.indirect_dma_start(
                            out=g_j[:, q, :], out_offset=None, in_=xcat_d[:],
                            in_offset=bass.IndirectOffsetOnAxis(
                                ap=idx[:, t:t + 1], axis=0))
                    xg.append(g_j)

                # ---- transpose x to feature-major in the gather shadow
                xT = bigp.tile([128, S * Lb], F16, tag="xT", bufs=1)
                for j in range(S):
                    xtp = psB.tile([128, Lb], F16, tag="xtp", bufs=2)
                    for q in range(Lb // 128):
                        nc.tensor.transpose(xtp[:, bass.ts(q, 128)],
                                            xg[j][:, q, 0:EMB], eyef[:])
                    nc.scalar.activation(xT[:, bass.ts(j, Lb)], xtp[:],
                                         AF.Identity)

                # ---- per-edge sums over the 8 member planes (PE, row-major)
                g8rm = spool.tile([128, Lb], F32, tag="g8rm")
                q8rm = spool.tile([128, Lb], F32, tag="q8rm")
                gp = psA.tile([128, Lb], F32, tag="gp")
                qp = psA.tile([128, Lb], F32, tag="qp")
                for j in range(S):
                    nc.tensor.matmul(gp[:], eyef[:], xg[j][:, 0:Lb // 128, 0:EMB],
                                     start=(j == 0), stop=(j == S - 1))
                for j in range(S):
                    nc.tensor.matmul(qp[:], eyef[:], xg[j][:, 0:Lb // 128, EMB:2 * EMB],
                                     start=(j == 0), stop=(j == S - 1))
                nc.scalar.activation(g8rm[:], gp[:], AF.Identity)
                nc.scalar.activation(q8rm[:], qp[:], AF.Identity)

                # transpose per-edge stats to feature-major [EMB, 512]
                g8tp = psA.tile([128, Lb], F32, tag="gp", name="g8tp")
                q8tp = psA.tile([128, Lb], F32, tag="qp", name="q8tp")
                for q in range(Lb // 128):
                    nc.tensor.transpose(g8tp[:, bass.ts(q, 128)],
                                        g8rm[:, bass.ts(q, 128)], eye32[:])
                    nc.tensor.transpose(q8tp[:, bass.ts(q, 128)],
                                        q8rm[:, bass.ts(q, 128)], eye32[:])
                g8s = spool.tile([128, Lb], F32, tag="g8s")
                nc.scalar.activation(g8s[:], g8tp[:], AF.Identity)

                # GraphNorm per-hyperedge affine: A = w / sqrt(var+eps)
                t1 = spool.tile([128, Lb], F32, tag="t1")
                nc.scalar.activation(t1[:], g8s[:], AF.Square)
                vx8 = spool.tile([128, Lb], F32, tag="vx8")
                nc.vector.scalar_tensor_tensor(vx8[:], t1[:], vecs[:, 6:7],
                                               q8tp[:], op0=OP.mult,
                                               op1=OP.add)
                vc = spool.tile([128, Lb], F32, tag="vc")
                nc.vector.tensor_scalar(vc[:], vx8[:], 0.0, 8.0 * EPS,
                                        op0=OP.max, op1=OP.add)
                ex = spool.tile([128, Lb], F32, tag="ex")
                nc.scalar.activation(ex[:], vc[:], AF.Abs_reciprocal_sqrt,
                                     scale=0.125)
                A = spool.tile([128, Lb], F16, tag="A")
                nc.vector.tensor_scalar(A[:], ex[:], wgv, None, op0=OP.mult)
                w8 = spool.tile([128, Lb], F16, tag="w8")
                nc.vector.scalar_tensor_tensor(w8[:], ex[:], wgv, g8s[:],
                                               op0=OP.mult, op1=OP.mult)
                u = spool.tile([128, Lb], F16, tag="u")
                nc.vector.tensor_scalar(u[:], w8[:], s8v, None, op0=OP.mult)

                # per-hyperedge C = u @ Wu + w8 @ Ww
                cp = psB.tile([128, Lb], F32, tag="cpspfp", name="cp")
                nc.tensor.matmul(cp[:], wu[:], u[:], start=True, stop=False)
                nc.tensor.matmul(cp[:], ww[:], w8[:], start=False, stop=True)
                cs = spool.tile([128, Lb], F16, tag="cs")
                nc.scalar.activation(cs[:], cp[:], AF.Identity, bias=cconv)

                # ---- apply A (broadcast over planes), cheb matmul ----
                z = bigp.tile([128, S * Lb], F16, tag="z", bufs=1)
                rhs = bigp.tile([128, S * Lb], F16, tag="rhs", bufs=1)
                nc.vector.tensor_tensor(
                    rhs[:].rearrange("p (j c) -> p j c", j=S),
                    xT[:].rearrange("p (j c) -> p j c", j=S),
                    A[:].unsqueeze(1).broadcast_to([128, S, Lb]),
                    op=OP.mult)
                for j in range(S):
                    vp = psB.tile([128, Lb], F32, tag="vp", bufs=2)
                    nc.tensor.matmul(vp[:], wx[:], rhs[:, bass.ts(j, Lb)],
                                     start=True, stop=True)
                    # egress + per-edge C (and c_const, folded into cs) add
                    nc.vector.tensor_tensor(z[:, bass.ts(j, Lb)], vp[:],
                                            cs[:], op=OP.add)

                zc = bigp.tile([128, S * Lb], F16, tag="zc", bufs=1)
                nc.vector.tensor_scalar(zc[:], z[:], 1.0, -1.0,
                                        op0=OP.min, op1=OP.max)

                # ---- poolings over the 8 planes ----
                pl = [zc[:, bass.ts(j, Lb)] for j in range(S)]
                mx = [spool.tile([128, Lb], F16, tag=f"mx{k}", name=f"mx{k}")
                      for k in range(4)]
                mn = [spool.tile([128, Lb], F16, tag=f"mn{k}", name=f"mn{k}")
                      for k in range(4)]
                for k in range(4):
                    nc.vector.tensor_tensor(mx[k][:], pl[2 * k], pl[2 * k + 1],
                                            op=OP.max)
                    nc.vector.tensor_tensor(mn[k][:], pl[2 * k], pl[2 * k + 1],
                                            op=OP.min)
                mx2a = spool.tile([128, Lb], F16, tag="mx2a")
                mx2b = spool.tile([128, Lb], F16, tag="mx2b")
                mn2a = spool.tile([128, Lb], F16, tag="mn2a")
                mn2b = spool.tile([128, Lb], F16, tag="mn2b")
                nc.vector.tensor_tensor(mx2a[:], mx[0][:], mx[1][:], op=OP.max)
                nc.vector.tensor_tensor(mx2b[:], mx[2][:], mx[3][:], op=OP.max)
                nc.vector.tensor_tensor(mn2a[:], mn[0][:], mn[1][:], op=OP.min)
                nc.vector.tensor_tensor(mn2b[:], mn[2][:], mn[3][:], op=OP.min)
                zmax = spool.tile([128, Lb], F16, tag="zmax")
                zmin = spool.tile([128, Lb], F16, tag="zmin")
                nc.vector.tensor_tensor(zmax[:], mx2a[:], mx2b[:], op=OP.max)
                nc.vector.tensor_tensor(zmin[:], mn2a[:], mn2b[:], op=OP.min)
                rng = spool.tile([128, Lb], F16, tag="rng")
                nc.vector.tensor_tensor(rng[:], zmax[:], zmin[:],
                                        op=OP.subtract)

                sq2 = bigp.tile([128, S * Lb], F16, tag="sq2", bufs=1)
                nc.scalar.activation(sq2[:], zc[:], AF.Square)
                sp = psB.tile([128, Lb], F32, tag="cpspfp", name="sp")
                for j in range(S):
                    nc.tensor.matmul(sp[:], eyef[:], sq2[:, bass.ts(j, Lb)],
                                     start=(j == 0), stop=(j == S - 1))
                # ynorm = sqrt(ssq/8) = (ssq/8) * rsqrt(ssq/8)
                r2 = spool.tile([128, Lb], F32, tag="r2")
                nc.scalar.activation(r2[:], sp[:], AF.Abs_reciprocal_sqrt,
                                     scale=0.125, bias=vecs[:, 5:6])
                ynorm = spool.tile([128, Lb], F16, tag="ynorm")
                nc.vector.scalar_tensor_tensor(ynorm[:], sp[:], 0.125, r2[:],
                                               op0=OP.mult, op1=OP.mult)

                fp = psB.tile([1, Lb], F32, tag="cpspfp", name="fp")
                nc.tensor.matmul(fp[:], wo[:, 0:1], rng[:],
                                 start=True, stop=False)
                nc.tensor.matmul(fp[:], wo[:, 1:2], ynorm[:],
                                 start=False, stop=True)
                nc.scalar.activation(logit[0:1, e0:e0 + Lb], fp[:],
                                     AF.Identity)

                tcol += S * (Lb // 128)

            ysb = wpool.tile([1, EPAD], F32, tag="ysb")
            nc.scalar.activation(ysb[:], logit[:], AF.Sigmoid, bias=boutv)
            nc.sync.dma_start(yout_d[:].rearrange("(p c) -> p c", p=1), ysb[:])

    nc.compile()
    return nc


def _get_program():
    if "nc" not in _CACHE:
        _CACHE["nc"] = _build_program()
    return _CACHE["nc"]


def _host_prep(inputs):
    """Fold weights and stage per-core input maps."""
    f = lambda k: np.asarray(inputs[k], np.float32)
    feature = f("feature")
    W_enc, b_enc = f("W_enc"), f("b_enc")
    gw, gb, gs = f("gn_weight"), f("gn_bias"), f("gn_mean_scale")
    cheb_W = np.asarray(inputs["cheb_W"], np.float64)
    cheb_b = np.asarray(inputs["cheb_b"], np.float64)
    W_out, b_out = f("W_out"), f("b_out")
    hn = np.asarray(inputs["hyperedge_nodes"]).astype(np.int64)

    d = float(S - 1)
    W0, W1, W2 = cheb_W[0], cheb_W[1], cheb_W[2]
    Wx64 = W0 + W1 / d + W2 * ((2.0 - d * d) / (d * d))
    Wg64 = -W1 / d + W2 * (2.0 * (d - 1.0) / (d * d))
    c_const = (gb.astype(np.float64) @ (Wx64 + S * Wg64) + cheb_b)

    featT = np.zeros((F, NPAD), np.float16)
    featT[:, :N] = feature.T.astype(np.float16)
    wenc = W_enc.astype(np.float16)
    benc = b_enc.reshape(1, EMB).astype(np.float16)
    wx16 = Wx64.astype(np.float16)
    wu16 = (-(Wx64 + S * Wg64)).astype(np.float16)
    ww16 = Wg64.astype(np.float16)
    wo16 = np.stack([W_out[:CONV, 0], W_out[CONV:, 0]], axis=1).astype(np.float16)
    eyef = np.eye(128, dtype=np.float16)
    eye32 = np.eye(128, dtype=np.float32)
    vecs = np.zeros((128, 8), np.float32)
    vecs[:, 0] = (2.0 * gs - gs * gs) / 8.0
    vecs[:, 1] = gw
    vecs[:, 2] = gs / 8.0
    vecs[:, 3] = c_const.astype(np.float32)
    vecs[0, 4] = b_out[0]
    vecs[:, 5] = 1e-30
    vecs[:, 6] = -(2.0 * gs - gs * gs) / 8.0

    shared = dict(featT=featT, wenc=wenc, benc=benc, wx=wx16, wu=wu16,
                  ww=ww16, wo=wo16, eyef=eyef, eye32=eye32, vecs=vecs)

    in_maps = []
    for c in range(NCORES):
        base = c * ECORE
        hcol = np.zeros((EPAD, S), np.int32)
        hcol[:ECORE] = hn[base:base + ECORE].astype(np.int32)
        # gather t = b*32 + j*4 + q covers edges [b*512+q*128, +128), member j
        idx = np.zeros((128, NG), np.int32)
        t = 0
        for e0, lb in BLOCKS:
            for j in range(S):
                for q in range(lb // 128):
                    idx[:, t] = hcol[e0 + q * 128:e0 + q * 128 + 128, j]
                    t += 1
        in_maps.append(dict(shared, idx32=idx))
    return in_maps


def _install_trace_hook():
    """Best-effort NTFF profiling under axon (test/benchmark only)."""
    import types
    ah = sys.modules.get("antenv.axon_hooks")
    if ah is None:
        ah = types.ModuleType("antenv.axon_hooks")
        ah._HOOK = None
        ah.set_axon_ntff_profile_hook = lambda h: setattr(ah, "_HOOK", h)
        ah.get_axon_ntff_profile_hook = lambda: ah._HOOK
        sys.modules["antenv.axon_hooks"] = ah
        import antenv
        antenv.axon_hooks = ah
    if ah.get_axon_ntff_profile_hook() is None:
        from trn_agent_boot.trn_boot import _ntff_profile_via_ctypes
        hook = _ntff_profile_via_ctypes("/opt/axon/libaxon_pjrt.so")
        if hook is not None:
            ah.set_axon_ntff_profile_hook(hook)
    import concourse.bass_utils as bu
    bu.upload_artifacts = lambda tmpdir: f"local:{tmpdir}"


def _run(in_maps, trace=False):
    nc = _get_program()
    if trace:
        _install_trace_hook()
    return run_bass_kernel_spmd(nc, in_maps, list(range(NCORES)), trace=trace)


def kernel(**inputs) -> np.ndarray:
    in_maps = _host_prep(inputs)
    res = _run(in_maps)
    out = np.concatenate([res.results[c]["yout"][:ECORE] for c in range(NCORES)])
    return out.reshape(E, 1).astype(np.float32)


def kernel_traced(**inputs):
    """Like kernel() but returns (output, exec_time_ns) using a profiled run."""
    in_maps = _host_prep(inputs)
    res = _run(in_maps, trace=True)
    out = np.concatenate([res.results[c]["yout"][:ECORE] for c in range(NCORES)])
    return out.reshape(E, 1).astype(np.float32), res.exec_time_ns



# revision 3
# speedup vs baseline: 1.3884x; 1.3884x over previous
"""Trainium2 Bass kernel for nn_CHESHIRE (hypergraph GNN message passing).

Strategy (hyperedge-parallel across the 8 cores):
  * Clique Laplacian over 8-node cliques collapses the K=3 Chebyshev conv to
    out = x_gn @ Wx + gsum(x_gn) @ Wg with host-folded weight combos; the
    GraphNorm affine x_gn = x*A_e + B_e is folded into the same matmuls.
  * Node encodings are computed once per core and stored to DRAM as an fp16
    [node, x] table (256B rows); incidence rows are fetched with SWDGE
    dma_gather (transpose=True), which lands the data feature-major directly
    (no PE transposes) at ~1 descriptor per row, chunked to fit the 64-entry
    per-DMA descriptor ring (<=896 idxs per gather), round-robined over 4
    SWDGE queues so descriptor generation overlaps DMA drain.
  * Per-edge sums (g8) accumulate over the 8 member planes in PSUM via
    identity matmuls; q8/ssq/max/min pool via fp16 tensor-tensor trees.
  * The per-edge ChebConv constant C is prefilled into PSUM (identity matmul)
    so the conv matmul accumulates on top; max/min/clip are reordered after
    the pooling trees (exact by monotonicity; sum pooling uses
    min(z^2,1) == clip(z)^2).
"""

import sys

sys.path.insert(0, "/opt/trn_rl_repo")

import numpy as np

import concourse.bacc as bacc
import concourse.bass as bass
import concourse.mybir as mybir
from concourse import tile
from concourse.bass_utils import run_bass_kernel_spmd

F16 = mybir.dt.float16
F32 = mybir.dt.float32
I16 = mybir.dt.int16
AF = mybir.ActivationFunctionType
OP = mybir.AluOpType

# Problem constants (hardcoded per contract).
N, F, EMB, CONV = 2000, 256, 128, 128
E, S = 20000, 8
NCORES = 8
ECORE = E // NCORES          # 2500
NBLK = 5
LB = 512                     # edges per block
EPAD = NBLK * LB             # 2560
COLS = S * LB                # 4096 gathered columns per block
# gather chunks per block (descriptor-ring limit: <=896 idxs per gather)
CHUNKS = [896, 896, 896, 896, 512]
assert sum(CHUNKS) == COLS
NIDXCOL = EPAD * S // 16     # 1280 int16 idx columns per core
NPAD = 2048                  # padded node count
EPS = 1e-5

_CACHE = {}


def _build_program():
    nc = bacc.Bacc(None, target_bir_lowering=False, debug=False,
                   num_swdge_queues=4)

    featT_d = nc.dram_tensor("featT", [F, NPAD], F16, kind="ExternalInput")
    wenc_d = nc.dram_tensor("wenc", [F, EMB], F16, kind="ExternalInput")
    wx_d = nc.dram_tensor("wx", [EMB, CONV], F16, kind="ExternalInput")
    wu_d = nc.dram_tensor("wu", [EMB, CONV], F16, kind="ExternalInput")
    ww_d = nc.dram_tensor("ww", [EMB, CONV], F16, kind="ExternalInput")
    wo_d = nc.dram_tensor("wo", [CONV, 4], F16, kind="ExternalInput")
    eyef_d = nc.dram_tensor("eyef", [128, 128], F16, kind="ExternalInput")
    vecs_d = nc.dram_tensor("vecs", [128, 8], F32, kind="ExternalInput")
    idx_d = nc.dram_tensor("idx16", [128, NIDXCOL], I16, kind="ExternalInput")
    yout_d = nc.dram_tensor("yout", [EPAD], F32, kind="ExternalOutput")

    xtab_d = nc.dram_tensor("xtab_scratch", [NPAD, EMB], F16)

    with tile.TileContext(nc) as tc:
        with (
            tc.tile_pool(name="weights", bufs=1) as wpool,
            tc.tile_pool(name="gath", bufs=2) as gpool,
            tc.tile_pool(name="sq", bufs=2) as qpool,
            tc.tile_pool(name="smalls", bufs=2) as spool,
            tc.tile_pool(name="psVP", bufs=2, space="PSUM") as psVP,
            tc.tile_pool(name="psCS", bufs=1, space="PSUM") as psCS,
            tc.tile_pool(name="psG8", bufs=1, space="PSUM") as psG8,
            tc.tile_pool(name="psFIN", bufs=1, space="PSUM") as psFIN,
        ):
            # ---- load weights / tables ----
            featT0 = wpool.tile([128, NPAD], F16, tag="featT0")
            featT1 = wpool.tile([128, NPAD], F16, tag="featT1")
            nc.sync.dma_start(featT0[:], featT_d[0:128, :])
            nc.sync.dma_start(featT1[:], featT_d[128:256, :])
            wenc0 = wpool.tile([128, EMB], F16, tag="wenc0")
            wenc1 = wpool.tile([128, EMB], F16, tag="wenc1")
            nc.sync.dma_start(wenc0[:], wenc_d[0:128, :])
            nc.sync.dma_start(wenc1[:], wenc_d[128:256, :])
            wx = wpool.tile([EMB, CONV], F16, tag="wx")
            nc.sync.dma_start(wx[:], wx_d[:])
            wu = wpool.tile([EMB, CONV], F16, tag="wu")
            nc.sync.dma_start(wu[:], wu_d[:])
            ww = wpool.tile([EMB, CONV], F16, tag="ww")
            nc.sync.dma_start(ww[:], ww_d[:])
            wo = wpool.tile([CONV, 4], F16, tag="wo")
            nc.sync.dma_start(wo[:], wo_d[:])
            eyef = wpool.tile([128, 128], F16, tag="eyef")
            nc.sync.dma_start(eyef[:], eyef_d[:])
            vecs = wpool.tile([128, 8], F32, tag="vecs")
            nc.sync.dma_start(vecs[:], vecs_d[:])
            idx = wpool.tile([128, NIDXCOL], I16, tag="idx")
            nc.sync.dma_start(idx[:], idx_d[:])

            wgv = vecs[:, 1:2]     # gn_weight
            s8v = vecs[:, 2:3]     # gn_mean_scale/8
            cconv = vecs[:, 3:4]   # c_const (+cheb_b) per CONV feature
            boutv = vecs[0:1, 4:5]  # b_out scalar
            bencv = vecs[:, 5:6]   # encoder bias per EMB feature
            v6 = vecs[:, 6:7]      # -(2s - s^2)/8

            # ---- encoder (feature-major) + transpose to node-major table ----
            # xe[emb, node] = clip(wenc.T @ featT + benc)
            xef = wpool.tile([128, NPAD], F16, tag="xef")
            for g in range(4):
                ep = psVP.tile([128, 512], F32, tag="vp", name="ep")
                nc.tensor.matmul(ep[:], wenc0[:], featT0[:, bass.ts(g, 512)],
                                 start=True, stop=False)
                nc.tensor.matmul(ep[:], wenc1[:], featT1[:, bass.ts(g, 512)],
                                 start=False, stop=True)
                # bias (per-partition) then clip
                eb = spool.tile([128, 512], F16, tag="eb", name=f"eb{g}")
                nc.scalar.activation(eb[:], ep[:], AF.Identity, bias=bencv)
                nc.vector.tensor_scalar(xef[:, bass.ts(g, 512)], eb[:],
                                        1.0, -1.0, op0=OP.min, op1=OP.max)
            # transpose 16 chunks to node-major and store rows
            for t in range(16):
                xtp = psCS.tile([128, 128], F16, tag="cs", name=f"xtp{t}")
                nc.tensor.transpose(xtp[:], xef[:, bass.ts(t, 128)], eyef[:])
                xnm = spool.tile([128, 128], F16, tag="xnm", name=f"xnm{t}")
                nc.scalar.activation(xnm[:], xtp[:], AF.Identity)
                nc.sync.dma_start(xtab_d[t * 128:(t + 1) * 128, :], xnm[:])

            logit = wpool.tile([1, EPAD], F32, tag="logit")

            for b in range(NBLK):
                # ---- gather 4096 incidence rows, feature-major ----
                xT = gpool.tile([128, COLS], F16, tag="xT", name=f"xT{b}")
                c0 = 0
                for t, ch in enumerate(CHUNKS):
                    gslice = xT[:, c0:c0 + ch].unsqueeze(1)
                    icol = (b * COLS + c0) // 16
                    nc.gpsimd.dma_gather(
                        gslice, xtab_d[:], idx[:, icol:icol + ch // 16],
                        ch, ch, EMB, transpose=True,
                        queue_num=t % 4)
                    c0 += ch

                xpl = xT[:].rearrange("p (j e) -> p j e", j=S)

                # ---- xsq = x^2 (scalar), q8 tree (vector, fp16) ----
                xsq = qpool.tile([128, S, LB], F16, tag="xsq", name=f"xsq{b}")
                nc.scalar.activation(xsq[:].rearrange("p j e -> p (j e)"),
                                     xT[:], AF.Square)
                q1 = spool.tile([128, 4, LB], F16, tag="q1")
                nc.vector.tensor_tensor(q1[:], xsq[:, 0:4, :], xsq[:, 4:8, :],
                                        op=OP.add)
                q2 = spool.tile([128, 2, LB], F16, tag="q2")
                nc.vector.tensor_tensor(q2[:], q1[:, 0:2, :], q1[:, 2:4, :],
                                        op=OP.add)
                q8 = spool.tile([128, LB], F32, tag="q8")
                nc.vector.tensor_tensor(q8[:], q2[:, 0, :], q2[:, 1, :],
                                        op=OP.add)

                # ---- g8 = sum over planes (PE identity-accumulate) ----
                gp = psG8.tile([128, LB], F32, tag="g8")
                for j in range(S):
                    nc.tensor.matmul(gp[:], eyef[:], xpl[:, j, :],
                                     start=(j == 0), stop=(j == S - 1))
                g8 = spool.tile([128, LB], F32, tag="g8s")
                nc.scalar.activation(g8[:], gp[:], AF.Identity)

                # ---- GraphNorm affine (fp32 chain): A = gw/sqrt(var+eps) ----
                t1 = spool.tile([128, LB], F32, tag="t1")
                nc.scalar.activation(t1[:], g8[:], AF.Square)
                vx8 = spool.tile([128, LB], F32, tag="vx8")
                nc.vector.scalar_tensor_tensor(vx8[:], t1[:], v6, q8[:],
                                               op0=OP.mult, op1=OP.add)
                vc = spool.tile([128, LB], F32, tag="vc")
                nc.vector.tensor_scalar(vc[:], vx8[:], 0.0, 8.0 * EPS,
                                        op0=OP.max, op1=OP.add)
                ex = spool.tile([128, LB], F32, tag="ex")
                nc.scalar.activation(ex[:], vc[:], AF.Abs_reciprocal_sqrt,
                                     scale=0.125)
                A = spool.tile([128, LB], F16, tag="A")
                nc.vector.tensor_scalar(A[:], ex[:], wgv, None, op0=OP.mult)
                w8 = spool.tile([128, LB], F16, tag="w8")
                nc.vector.tensor_tensor(w8[:], A[:], g8[:], op=OP.mult)
                u = spool.tile([128, LB], F16, tag="u")
                nc.vector.tensor_scalar(u[:], w8[:], s8v, None, op0=OP.mult)

                # ---- per-hyperedge C = u @ Wu + w8 @ Ww (+cconv) ----
                cp = psCS.tile([128, LB], F32, tag="cs", name=f"cp{b}")
                nc.tensor.matmul(cp[:], wu[:], u[:], start=True, stop=False)
                nc.tensor.matmul(cp[:], ww[:], w8[:], start=False, stop=True)
                cs = spool.tile([128, LB], F16, tag="cs16")
                nc.scalar.activation(cs[:], cp[:], AF.Identity, bias=cconv)

                # ---- rhs = x * A (broadcast over planes) ----
                rhs = gpool.tile([128, S, LB], F16, tag="rhs", name=f"rhs{b}")
                nc.vector.tensor_tensor(
                    rhs[:], xpl,
                    A[:].unsqueeze(1).broadcast_to([128, S, LB]), op=OP.mult)

                # ---- cheb conv in 4 waves of 2 planes; pool trees stream ----
                mx = [spool.tile([128, LB], F16, tag=f"mx{k}", name=f"mx{k}")
                      for k in range(2)]
                mn = [spool.tile([128, LB], F16, tag=f"mn{k}", name=f"mn{k}")
                      for k in range(2)]
                sa = [spool.tile([128, LB], F16, tag=f"sa{k}", name=f"sa{k}")
                      for k in range(2)]
                for w in range(4):
                    vp = psVP.tile([128, 2, LB], F32, tag="vp", name=f"vp{b}_{w}")
                    for h in range(2):
                        j = 2 * w + h
                        nc.tensor.matmul(vp[:, h, :], eyef[:], cs[:],
                                         start=True, stop=False)
                        nc.tensor.matmul(vp[:, h, :], wx[:], rhs[:, j, :],
                                         start=False, stop=True)
                    # egress both planes fp32->fp16 (scalar)
                    z2 = qpool.tile([128, 2, LB], F16, tag="z2", name=f"z2_{b}_{w}")
                    nc.scalar.activation(z2[:].rearrange("p j e -> p (j e)"),
                                         vp[:].rearrange("p j e -> p (j e)"),
                                         AF.Identity)
                    # wave-level max/min; stream into accumulators
                    wmx = spool.tile([128, LB], F16, tag="wmx", name=f"wmx{w}")
                    nc.vector.tensor_tensor(wmx[:], z2[:, 0, :], z2[:, 1, :],
                                            op=OP.max)
                    wmn = spool.tile([128, LB], F16, tag="wmn", name=f"wmn{w}")
                    nc.vector.tensor_tensor(wmn[:], z2[:, 0, :], z2[:, 1, :],
                                            op=OP.min)
                    # squares (DVE, fp16 2x), min-with-1, wave sum
                    zq = spool.tile([128, 2, LB], F16, tag="zq", name=f"zq{w}")
                    nc.vector.tensor_tensor(zq[:], z2[:], z2[:], op=OP.mult)
                    zqc = spool.tile([128, 2, LB], F16, tag="zqc", name=f"zqc{w}")
                    nc.vector.tensor_scalar(zqc[:], zq[:], 1.0, None,
                                            op0=OP.min)
                    wsq = spool.tile([128, LB], F16, tag="wsq", name=f"wsq{w}")
                    nc.vector.tensor_tensor(wsq[:], zqc[:, 0, :], zqc[:, 1, :],
                                            op=OP.add)
                    k = w // 2
                    if w % 2 == 0:
                        nc.scalar.activation(mx[k][:], wmx[:], AF.Identity)
                        nc.scalar.activation(mn[k][:], wmn[:], AF.Identity)
                        nc.scalar.activation(sa[k][:], wsq[:], AF.Identity)
                    else:
                        nc.vector.tensor_tensor(mx[k][:], mx[k][:], wmx[:],
                                                op=OP.max)
                        nc.vector.tensor_tensor(mn[k][:], mn[k][:], wmn[:],
                                                op=OP.min)
                        nc.vector.tensor_tensor(sa[k][:], sa[k][:], wsq[:],
                                                op=OP.add)

                zmax = spool.tile([128, LB], F16, tag="zmax")
                nc.vector.tensor_tensor(zmax[:], mx[0][:], mx[1][:], op=OP.max)
                zmin = spool.tile([128, LB], F16, tag="zmin")
                nc.vector.tensor_tensor(zmin[:], mn[0][:], mn[1][:], op=OP.min)
                ssq = spool.tile([128, LB], F16, tag="ssq")
                nc.vector.tensor_tensor(ssq[:], sa[0][:], sa[1][:], op=OP.add)

                # clip pooled extrema (exact: clip commutes with max/min)
                zmaxc = spool.tile([128, LB], F16, tag="zmaxc")
                nc.vector.tensor_scalar(zmaxc[:], zmax[:], 1.0, -1.0,
                                        op0=OP.min, op1=OP.max)
                zminc = spool.tile([128, LB], F16, tag="zminc")
                nc.vector.tensor_scalar(zminc[:], zmin[:], 1.0, -1.0,
                                        op0=OP.min, op1=OP.max)
                # ynorm = sqrt(ssq/8)
                ynorm = spool.tile([128, LB], F16, tag="ynorm")
                nc.scalar.activation(ynorm[:], ssq[:], AF.Sqrt, scale=0.125)

                # ---- logits: wo0@zmaxc - wo0@zminc + wo1@ynorm ----
                fp = psFIN.tile([1, LB], F32, tag="fin")
                nc.tensor.matmul(fp[:], wo[:, 0:1], zmaxc[:],
                                 start=True, stop=False)
                nc.tensor.matmul(fp[:], wo[:, 1:2], zminc[:],
                                 start=False, stop=False)
                nc.tensor.matmul(fp[:], wo[:, 2:3], ynorm[:],
                                 start=False, stop=True)
                nc.scalar.activation(logit[0:1, b * LB:(b + 1) * LB], fp[:],
                                     AF.Identity)

            ysb = wpool.tile([1, EPAD], F32, tag="ysb")
            nc.scalar.activation(ysb[:], logit[:], AF.Sigmoid, bias=boutv)
            nc.sync.dma_start(yout_d[:].rearrange("(p c) -> p c", p=1), ysb[:])

    nc.compile()
    return nc


def _get_program():
    if "nc" not in _CACHE:
        _CACHE["nc"] = _build_program()
    return _CACHE["nc"]


def _host_prep(inputs):
    """Fold weights and stage per-core input maps."""
    f = lambda k: np.asarray(inputs[k], np.float32)
    feature = f("feature")
    W_enc, b_enc = f("W_enc"), f("b_enc")
    gw, gb, gs = f("gn_weight"), f("gn_bias"), f("gn_mean_scale")
    cheb_W = np.asarray(inputs["cheb_W"], np.float64)
    cheb_b = np.asarray(inputs["cheb_b"], np.float64)
    W_out, b_out = f("W_out"), f("b_out")
    hn = np.asarray(inputs["hyperedge_nodes"]).astype(np.int64)

    d = float(S - 1)
    W0, W1, W2 = cheb_W[0], cheb_W[1], cheb_W[2]
    Wx64 = W0 + W1 / d + W2 * ((2.0 - d * d) / (d * d))
    Wg64 = -W1 / d + W2 * (2.0 * (d - 1.0) / (d * d))
    c_const = (gb.astype(np.float64) @ (Wx64 + S * Wg64) + cheb_b)

    featT = np.zeros((F, NPAD), np.float16)
    featT[:, :N] = feature.T.astype(np.float16)
    wenc = W_enc.astype(np.float16)
    wx16 = Wx64.astype(np.float16)
    wu16 = (-(Wx64 + S * Wg64)).astype(np.float16)
    ww16 = Wg64.astype(np.float16)
    # wo columns: [wo_max, -wo_max (for zmin), wo_norm, unused]
    wo16 = np.zeros((CONV, 4), np.float16)
    wo16[:, 0] = W_out[:CONV, 0].astype(np.float16)
    wo16[:, 1] = (-W_out[:CONV, 0]).astype(np.float16)
    wo16[:, 2] = W_out[CONV:, 0].astype(np.float16)
    eyef = np.eye(128, dtype=np.float16)
    vecs = np.zeros((128, 8), np.float32)
    vecs[:, 1] = gw
    vecs[:, 2] = gs / 8.0
    vecs[:, 3] = c_const.astype(np.float32)
    vecs[0, 4] = b_out[0]
    vecs[:, 5] = b_enc
    vecs[:, 6] = -(2.0 * gs - gs * gs) / 8.0

    shared = dict(featT=featT, wenc=wenc, wx=wx16, wu=wu16,
                  ww=ww16, wo=wo16, eyef=eyef, vecs=vecs)

    in_maps = []
    for c in range(NCORES):
        base = c * ECORE
        hpad = np.zeros((EPAD, S), np.int16)
        hpad[:ECORE] = hn[base:base + ECORE].astype(np.int16)
        # per block b: column i = j*LB + e -> node hpad[b*LB + e, j]
        # wrapped: position k of the block -> partition k%16, idxcol k//16
        unwrapped = hpad.reshape(NBLK, LB, S).transpose(0, 2, 1).reshape(-1)
        idx16 = np.zeros((128, NIDXCOL), np.int16)
        wrapped = unwrapped.reshape(NIDXCOL, 16).T  # [16, NIDXCOL]
        for r in range(8):
            idx16[16 * r:16 * r + 16] = wrapped
        in_maps.append(dict(shared, idx16=idx16))
    return in_maps


def _install_trace_hook():
    """Best-effort NTFF profiling under axon (test/benchmark only)."""
    import types
    ah = sys.modules.get("antenv.axon_hooks")
    if ah is None:
        ah = types.ModuleType("antenv.axon_hooks")
        ah._HOOK = None
        ah.set_axon_ntff_profile_hook = lambda h: setattr(ah, "_HOOK", h)
        ah.get_axon_ntff_profile_hook = lambda: ah._HOOK
        sys.modules["antenv.axon_hooks"] = ah
        import antenv
        antenv.axon_hooks = ah
    if ah.get_axon_ntff_profile_hook() is None:
        from trn_agent_boot.trn_boot import _ntff_profile_via_ctypes
        hook = _ntff_profile_via_ctypes("/opt/axon/libaxon_pjrt.so")
        if hook is not None:
            ah.set_axon_ntff_profile_hook(hook)
    import concourse.bass_utils as bu
    bu.upload_artifacts = lambda tmpdir: f"local:{tmpdir}"


def _run(in_maps, trace=False):
    nc = _get_program()
    if trace:
        _install_trace_hook()
    return run_bass_kernel_spmd(nc, in_maps, list(range(NCORES)), trace=trace)


def kernel(**inputs) -> np.ndarray:
    in_maps = _host_prep(inputs)
    res = _run(in_maps)
    out = np.concatenate([res.results[c]["yout"][:ECORE] for c in range(NCORES)])
    return out.reshape(E, 1).astype(np.float32)


def kernel_traced(**inputs):
    """Like kernel() but returns (output, exec_time_ns) using a profiled run."""
    in_maps = _host_prep(inputs)
    res = _run(in_maps, trace=True)
    out = np.concatenate([res.results[c]["yout"][:ECORE] for c in range(NCORES)])
    return out.reshape(E, 1).astype(np.float32), res.exec_time_ns


# revision 6
# speedup vs baseline: 1.8118x; 1.3049x over previous
"""Trainium2 Bass kernel for nn_CHESHIRE (hypergraph GNN message passing).

Strategy (hyperedge-parallel across the 8 cores):
  * Clique Laplacian over 8-node cliques collapses the K=3 Chebyshev conv to
    out = x_gn @ Wx + gsum(x_gn) @ Wg with host-folded weight combos; the
    GraphNorm affine is folded into the same matmuls, with gn_weight/
    gn_mean_scale folded into the weight matrices host-side so the per-edge
    scale is just ex = rsqrt(var+eps).
  * Node encodings are computed once per core and stored to DRAM as an fp16
    [node, x] table (256B rows); incidence rows are fetched with SWDGE
    dma_gather (transpose=True), which lands the data feature-major directly
    (no PE transposes), one 512-idx gather per member plane, round-robined
    over 4 SWDGE queues so descriptor rings never block back-to-back.
  * Per-edge sums (g8) accumulate over the 8 member planes in PSUM via
    identity matmuls; the stats chain runs in fp32 (variance cancellation);
    q8/ssq/max/min pool via fp16 tensor-tensor trees.
  * The per-edge ChebConv constant C is prefilled into PSUM (identity matmul)
    so the conv matmul accumulates on top; clip is applied after the pooling
    trees (exact: clip is monotone, and min(z^2,1) == clip(z)^2).
"""

import sys

sys.path.insert(0, "/opt/trn_rl_repo")

import numpy as np

import concourse.bacc as bacc
import concourse.bass as bass
import concourse.mybir as mybir
from concourse import tile
from concourse.bass_utils import run_bass_kernel_spmd

F16 = mybir.dt.float16
F32 = mybir.dt.float32
I16 = mybir.dt.int16
AF = mybir.ActivationFunctionType
OP = mybir.AluOpType

# Problem constants (hardcoded per contract).
N, F, EMB, CONV = 2000, 256, 128, 128
E, S = 20000, 8
NCORES = 8
ECORE = E // NCORES          # 2500
NBLK = 5
LB = 512                     # edges per block
EPAD = NBLK * LB             # 2560
COLS = S * LB                # 4096 gathered columns per block
NIDXCOL = EPAD * S // 16     # 1280 int16 idx columns per core
NPAD = 2048                  # padded node count
EPS = 1e-5

_CACHE = {}


def _build_program():
    nc = bacc.Bacc(None, target_bir_lowering=False, debug=False,
                   num_swdge_queues=4)

    featT_d = nc.dram_tensor("featT", [F, NPAD], F16, kind="ExternalInput")
    wenc_d = nc.dram_tensor("wenc", [F, EMB], F16, kind="ExternalInput")
    wx_d = nc.dram_tensor("wx", [EMB, CONV], F16, kind="ExternalInput")
    wcs_d = nc.dram_tensor("wcs", [EMB, CONV], F16, kind="ExternalInput")
    wo_d = nc.dram_tensor("wo", [CONV, 4], F16, kind="ExternalInput")
    eyef_d = nc.dram_tensor("eyef", [128, 128], F16, kind="ExternalInput")
    vecs_d = nc.dram_tensor("vecs", [128, 8], F32, kind="ExternalInput")
    idx_d = nc.dram_tensor("idx16", [128, NIDXCOL], I16, kind="ExternalInput")
    yout_d = nc.dram_tensor("yout", [EPAD], F32, kind="ExternalOutput")

    xtab_d = nc.dram_tensor("xtab_scratch", [NPAD, EMB], F16)

    with tile.TileContext(nc) as tc:
        with (
            tc.tile_pool(name="weights", bufs=1) as wpool,
            tc.tile_pool(name="gath", bufs=3) as gpool,
            tc.tile_pool(name="mid", bufs=2) as qpool,
            tc.tile_pool(name="smalls", bufs=1) as spool,
            tc.tile_pool(name="psVP", bufs=2, space="PSUM") as psVP,
            tc.tile_pool(name="psCS", bufs=1, space="PSUM") as psCS,
            tc.tile_pool(name="psG8", bufs=1, space="PSUM") as psG8,
            tc.tile_pool(name="psFIN", bufs=1, space="PSUM") as psFIN,
        ):
            # ---- load weights / tables ----
            featT0 = wpool.tile([128, NPAD], F16, tag="featT0")
            featT1 = wpool.tile([128, NPAD], F16, tag="featT1")
            nc.sync.dma_start(featT0[:], featT_d[0:128, :])
            nc.sync.dma_start(featT1[:], featT_d[128:256, :])
            wenc0 = wpool.tile([128, EMB], F16, tag="wenc0")
            wenc1 = wpool.tile([128, EMB], F16, tag="wenc1")
            nc.sync.dma_start(wenc0[:], wenc_d[0:128, :])
            nc.sync.dma_start(wenc1[:], wenc_d[128:256, :])
            wx = wpool.tile([EMB, CONV], F16, tag="wx")
            nc.sync.dma_start(wx[:], wx_d[:])
            wcs = wpool.tile([EMB, CONV], F16, tag="wcs")
            nc.sync.dma_start(wcs[:], wcs_d[:])
            wo = wpool.tile([CONV, 4], F16, tag="wo")
            nc.sync.dma_start(wo[:], wo_d[:])
            eyef = wpool.tile([128, 128], F16, tag="eyef")
            nc.sync.dma_start(eyef[:], eyef_d[:])
            vecs = wpool.tile([128, 8], F32, tag="vecs")
            nc.sync.dma_start(vecs[:], vecs_d[:])
            idx = wpool.tile([128, NIDXCOL], I16, tag="idx")
            nc.sync.dma_start(idx[:], idx_d[:])

            cconv = vecs[:, 3:4]   # c_const (+cheb_b) per CONV feature
            boutv = vecs[0:1, 4:5]  # b_out scalar
            bencv = vecs[:, 5:6]   # encoder bias per EMB feature
            v6 = vecs[:, 6:7]      # -(2s - s^2)/8

            # ---- encoder (feature-major) + transpose to node-major table ----
            xef = wpool.tile([128, NPAD], F16, tag="xef")
            for g in range(4):
                ep = psVP.tile([128, 512], F32, tag="vp", name="ep")
                nc.tensor.matmul(ep[:], wenc0[:], featT0[:, bass.ts(g, 512)],
                                 start=True, stop=False)
                nc.tensor.matmul(ep[:], wenc1[:], featT1[:, bass.ts(g, 512)],
                                 start=False, stop=True)
                eb = spool.tile([128, 512], F16, tag="eb", name=f"eb{g}",
                                bufs=2)
                nc.scalar.activation(eb[:], ep[:], AF.Identity, bias=bencv)
                nc.vector.tensor_scalar(xef[:, bass.ts(g, 512)], eb[:],
                                        1.0, -1.0, op0=OP.min, op1=OP.max)
            # transpose 16 chunks to node-major and store table rows
            for t in range(16):
                xtp = psVP.tile([128, 128], F16, tag="vp", name=f"xtp{t}")
                nc.tensor.transpose(xtp[:], xef[:, bass.ts(t, 128)], eyef[:])
                xnm = spool.tile([128, 128], F16, tag="xnm", name=f"xnm{t}",
                                 bufs=4)
                nc.scalar.activation(xnm[:], xtp[:], AF.Identity)
                nc.sync.dma_start(xtab_d[t * 128:(t + 1) * 128, :], xnm[:])

            logit = wpool.tile([1, EPAD], F32, tag="logit")
            gq = 0  # global gather counter for queue round-robin

            for b in range(NBLK):
                # ---- gather 4096 incidence rows, feature-major ----
                xT = gpool.tile([128, COLS], F16, tag="xT", name=f"xT{b}")
                for j in range(S):
                    gslice = xT[:, j * LB:(j + 1) * LB].unsqueeze(1)
                    icol = b * (COLS // 16) + j * (LB // 16)
                    nc.gpsimd.dma_gather(
                        gslice, xtab_d[:], idx[:, icol:icol + LB // 16],
                        LB, LB, EMB, transpose=True, queue_num=gq % 4)
                    gq += 1

                xpl = xT[:].rearrange("p (j e) -> p j e", j=S)

                # ---- xsq = x^2 (scalar), q8 tree (vector) ----
                xsq = qpool.tile([128, S, LB], F16, tag="xsq", name=f"xsq{b}")
                nc.scalar.activation(xsq[:].rearrange("p j e -> p (j e)"),
                                     xT[:], AF.Square)
                q1 = spool.tile([128, 4, LB], F16, tag="q1")
                nc.vector.tensor_tensor(q1[:], xsq[:, 0:4, :], xsq[:, 4:8, :],
                                        op=OP.add)
                q2 = spool.tile([128, 2, LB], F16, tag="q2")
                nc.vector.tensor_tensor(q2[:], q1[:, 0:2, :], q1[:, 2:4, :],
                                        op=OP.add)
                q8 = spool.tile([128, LB], F32, tag="q8")
                nc.vector.tensor_tensor(q8[:], q2[:, 0, :], q2[:, 1, :],
                                        op=OP.add)

                # ---- g8 = sum over planes (PE identity-accumulate) ----
                gp = psG8.tile([128, LB], F32, tag="g8")
                for j in range(S):
                    nc.tensor.matmul(gp[:], eyef[:], xpl[:, j, :],
                                     start=(j == 0), stop=(j == S - 1))
                g8 = spool.tile([128, LB], F32, tag="g8s")
                nc.scalar.activation(g8[:], gp[:], AF.Identity)

                # ---- GraphNorm scale (fp32 chain): ex = rsqrt(var+eps) ----
                t1 = spool.tile([128, LB], F32, tag="t1")
                nc.scalar.activation(t1[:], g8[:], AF.Square)
                vx8 = spool.tile([128, LB], F32, tag="vx8")
                nc.vector.scalar_tensor_tensor(vx8[:], t1[:], v6, q8[:],
                                               op0=OP.mult, op1=OP.add)
                vc = spool.tile([128, LB], F32, tag="vc")
                nc.vector.tensor_scalar(vc[:], vx8[:], 0.0, 8.0 * EPS,
                                        op0=OP.max, op1=OP.add)
                ex = spool.tile([128, LB], F16, tag="ex")
                nc.scalar.activation(ex[:], vc[:], AF.Abs_reciprocal_sqrt,
                                     scale=0.125)
                w8 = spool.tile([128, LB], F16, tag="w8")
                nc.vector.tensor_tensor(w8[:], ex[:], g8[:], op=OP.mult)

                # ---- per-hyperedge C = w8 @ Wcs (+cconv) ----
                cp = psCS.tile([128, LB], F32, tag="cs", name=f"cp{b}")
                nc.tensor.matmul(cp[:], wcs[:], w8[:], start=True, stop=True)
                cs = spool.tile([128, LB], F16, tag="cs16")
                nc.scalar.activation(cs[:], cp[:], AF.Identity, bias=cconv)

                # ---- rhs = x * ex (broadcast over planes) ----
                rhs = gpool.tile([128, S, LB], F16, tag="rhs", name=f"rhs{b}",
                                 bufs=2)
                nc.vector.tensor_tensor(
                    rhs[:], xpl,
                    ex[:].unsqueeze(1).broadcast_to([128, S, LB]), op=OP.mult)

                # ---- cheb conv in 4 waves of 2 planes (cs prefilled) ----
                z2 = qpool.tile([128, S, LB], F16, tag="z2", name=f"z2_{b}")
                for w in range(4):
                    vp = psVP.tile([128, 2, LB], F32, tag="vp",
                                   name=f"vp{b}_{w}")
                    for h in range(2):
                        j = 2 * w + h
                        nc.tensor.matmul(vp[:, h, :], eyef[:], cs[:],
                                         start=True, stop=False)
                        nc.tensor.matmul(vp[:, h, :], wx[:], rhs[:, j, :],
                                         start=False, stop=True)
                    nc.scalar.activation(
                        z2[:, 2 * w:2 * w + 2, :].rearrange(
                            "p j e -> p (j e)"),
                        vp[:].rearrange("p j e -> p (j e)"), AF.Identity)

                # ---- poolings: max/min/ssq trees (fp16) ----
                m1 = spool.tile([128, 4, LB], F16, tag="m1")
                nc.vector.tensor_tensor(m1[:], z2[:, 0:4, :], z2[:, 4:8, :],
                                        op=OP.max)
                m2 = spool.tile([128, 2, LB], F16, tag="m2")
                nc.vector.tensor_tensor(m2[:], m1[:, 0:2, :], m1[:, 2:4, :],
                                        op=OP.max)
                zmax = spool.tile([128, LB], F16, tag="zmax")
                nc.vector.tensor_tensor(zmax[:], m2[:, 0, :], m2[:, 1, :],
                                        op=OP.max)
                n1 = spool.tile([128, 4, LB], F16, tag="n1")
                nc.vector.tensor_tensor(n1[:], z2[:, 0:4, :], z2[:, 4:8, :],
                                        op=OP.min)
                n2 = spool.tile([128, 2, LB], F16, tag="n2")
                nc.vector.tensor_tensor(n2[:], n1[:, 0:2, :], n1[:, 2:4, :],
                                        op=OP.min)
                zmin = spool.tile([128, LB], F16, tag="zmin")
                nc.vector.tensor_tensor(zmin[:], n2[:, 0, :], n2[:, 1, :],
                                        op=OP.min)

                zq = qpool.tile([128, S, LB], F16, tag="zq", name=f"zq{b}",
                                bufs=1)
                nc.scalar.activation(zq[:].rearrange("p j e -> p (j e)"),
                                     z2[:].rearrange("p j e -> p (j e)"),
                                     AF.Square)
                zqc = qpool.tile([128, S, LB], F16, tag="zqc", name=f"zqc{b}",
                                 bufs=1)
                nc.vector.tensor_scalar(zqc[:], zq[:], 1.0, None, op0=OP.min)
                s1 = spool.tile([128, 4, LB], F16, tag="s1")
                nc.vector.tensor_tensor(s1[:], zqc[:, 0:4, :], zqc[:, 4:8, :],
                                        op=OP.add)
                s2 = spool.tile([128, 2, LB], F16, tag="s2")
                nc.vector.tensor_tensor(s2[:], s1[:, 0:2, :], s1[:, 2:4, :],
                                        op=OP.add)
                ssq = spool.tile([128, LB], F16, tag="ssq")
                nc.vector.tensor_tensor(ssq[:], s2[:, 0, :], s2[:, 1, :],
                                        op=OP.add)

                # clip pooled extrema (exact: clip commutes with max/min)
                zmaxc = spool.tile([128, LB], F16, tag="zmaxc")
                nc.vector.tensor_scalar(zmaxc[:], zmax[:], 1.0, -1.0,
                                        op0=OP.min, op1=OP.max)
                zminc = spool.tile([128, LB], F16, tag="zminc")
                nc.vector.tensor_scalar(zminc[:], zmin[:], 1.0, -1.0,
                                        op0=OP.min, op1=OP.max)
                ynorm = spool.tile([128, LB], F16, tag="ynorm")
                nc.scalar.activation(ynorm[:], ssq[:], AF.Sqrt, scale=0.125)

                # ---- logits: wo0@zmaxc - wo0@zminc + wo1@ynorm ----
                fp = psFIN.tile([1, LB], F32, tag="fin")
                nc.tensor.matmul(fp[:], wo[:, 0:1], zmaxc[:],
                                 start=True, stop=False)
                nc.tensor.matmul(fp[:], wo[:, 1:2], zminc[:],
                                 start=False, stop=False)
                nc.tensor.matmul(fp[:], wo[:, 2:3], ynorm[:],
                                 start=False, stop=True)
                nc.scalar.activation(logit[0:1, b * LB:(b + 1) * LB], fp[:],
                                     AF.Identity)

            ysb = wpool.tile([1, EPAD], F32, tag="ysb")
            nc.scalar.activation(ysb[:], logit[:], AF.Sigmoid, bias=boutv)
            nc.sync.dma_start(yout_d[:].rearrange("(p c) -> p c", p=1), ysb[:])

    nc.compile()
    return nc


def _get_program():
    if "nc" not in _CACHE:
        _CACHE["nc"] = _build_program()
    return _CACHE["nc"]


def _host_prep(inputs):
    """Fold weights and stage per-core input maps."""
    f = lambda k: np.asarray(inputs[k], np.float32)
    feature = f("feature")
    W_enc, b_enc = f("W_enc"), f("b_enc")
    gw, gb, gs = f("gn_weight"), f("gn_bias"), f("gn_mean_scale")
    cheb_W = np.asarray(inputs["cheb_W"], np.float64)
    cheb_b = np.asarray(inputs["cheb_b"], np.float64)
    W_out, b_out = f("W_out"), f("b_out")
    hn = np.asarray(inputs["hyperedge_nodes"]).astype(np.int64)

    d = float(S - 1)
    W0, W1, W2 = cheb_W[0], cheb_W[1], cheb_W[2]
    Wx64 = W0 + W1 / d + W2 * ((2.0 - d * d) / (d * d))
    Wg64 = -W1 / d + W2 * (2.0 * (d - 1.0) / (d * d))
    c_const = (gb.astype(np.float64) @ (Wx64 + S * Wg64) + cheb_b)
    gw64 = gw.astype(np.float64)[:, None]
    gs64 = gs.astype(np.float64)[:, None]
    # fold gn_weight into Wx; fold gn_weight * (u,w8) combo into one Wcs
    wx16 = (gw64 * Wx64).astype(np.float16)
    wcs16 = (gw64 * (gs64 / 8.0 * (-(Wx64 + S * Wg64)) + Wg64)).astype(
        np.float16)

    featT = np.zeros((F, NPAD), np.float16)
    featT[:, :N] = feature.T.astype(np.float16)
    wenc = W_enc.astype(np.float16)
    wo16 = np.zeros((CONV, 4), np.float16)
    wo16[:, 0] = W_out[:CONV, 0].astype(np.float16)
    wo16[:, 1] = (-W_out[:CONV, 0]).astype(np.float16)
    wo16[:, 2] = W_out[CONV:, 0].astype(np.float16)
    eyef = np.eye(128, dtype=np.float16)
    vecs = np.zeros((128, 8), np.float32)
    vecs[:, 3] = c_const.astype(np.float32)
    vecs[0, 4] = b_out[0]
    vecs[:, 5] = b_enc
    vecs[:, 6] = -(2.0 * gs - gs * gs) / 8.0

    shared = dict(featT=featT, wenc=wenc, wx=wx16, wcs=wcs16, wo=wo16,
                  eyef=eyef, vecs=vecs)

    in_maps = []
    for c in range(NCORES):
        base = c * ECORE
        hpad = np.zeros((EPAD, S), np.int16)
        hpad[:ECORE] = hn[base:base + ECORE].astype(np.int16)
        # per block b: column i = j*LB + e -> node hpad[b*LB + e, j]
        unwrapped = hpad.reshape(NBLK, LB, S).transpose(0, 2, 1).reshape(-1)
        idx16 = np.zeros((128, NIDXCOL), np.int16)
        wrapped = unwrapped.reshape(NIDXCOL, 16).T  # [16, NIDXCOL]
        for r in range(8):
            idx16[16 * r:16 * r + 16] = wrapped
        in_maps.append(dict(shared, idx16=idx16))
    return in_maps


def _install_trace_hook():
    """Best-effort NTFF profiling under axon (test/benchmark only)."""
    import types
    ah = sys.modules.get("antenv.axon_hooks")
    if ah is None:
        ah = types.ModuleType("antenv.axon_hooks")
        ah._HOOK = None
        ah.set_axon_ntff_profile_hook = lambda h: setattr(ah, "_HOOK", h)
        ah.get_axon_ntff_profile_hook = lambda: ah._HOOK
        sys.modules["antenv.axon_hooks"] = ah
        import antenv
        antenv.axon_hooks = ah
    if ah.get_axon_ntff_profile_hook() is None:
        from trn_agent_boot.trn_boot import _ntff_profile_via_ctypes
        hook = _ntff_profile_via_ctypes("/opt/axon/libaxon_pjrt.so")
        if hook is not None:
            ah.set_axon_ntff_profile_hook(hook)
    import concourse.bass_utils as bu
    bu.upload_artifacts = lambda tmpdir: f"local:{tmpdir}"


def _run(in_maps, trace=False):
    nc = _get_program()
    if trace:
        _install_trace_hook()
    return run_bass_kernel_spmd(nc, in_maps, list(range(NCORES)), trace=trace)


def kernel(**inputs) -> np.ndarray:
    in_maps = _host_prep(inputs)
    res = _run(in_maps)
    out = np.concatenate([res.results[c]["yout"][:ECORE] for c in range(NCORES)])
    return out.reshape(E, 1).astype(np.float32)


def kernel_traced(**inputs):
    """Like kernel() but returns (output, exec_time_ns) using a profiled run."""
    in_maps = _host_prep(inputs)
    res = _run(in_maps, trace=True)
    out = np.concatenate([res.results[c]["yout"][:ECORE] for c in range(NCORES)])
    return out.reshape(E, 1).astype(np.float32), res.exec_time_ns
